# revision 16
# baseline (speedup 1.0000x reference)
"""Trainium2 Bass kernel for nn_Attention_40716289966507.

Reference computation (B=4, C=256, H=W=48, heads=8, d=32, N=H*W=2304):
    qkv = w_qkv @ x            # 1x1 conv -> q,k,v each [B, 256, N]
    attn = softmax(q^T k / sqrt(d))   per (batch, head): [N, N]
    out  = v @ attn^T          # [B, 256, N]
    y    = w_proj @ out + b    # [B, 256, N]

Sharding (8 cores): core i handles batch b = i//2 and query-token half
t = i%2 (1152 of the 2304 tokens). Each core needs the full image of its
batch (for K and V) but only its token half for Q; it produces the full
256-channel output for its 1152 tokens, so the host just concatenates —
no cross-core reduction.

Per-core device pipeline (engine-parallel, ACT-bound by design):
  * qkv matmuls in float32r (full-rate fp32-ish); q/k stay f32r for the
    logit matmul's precision, v is materialized TRANSPOSED ([tokens,
    channels], fp16) directly off the tensor engine so the AV matmul
    needs no runtime transposes.
  * Attention per 4-head group, per query tile (512/512/128), per
    128-key chunk:
      - S^T[keys, q] via 4 row-packed f32r matmuls (K=32 at PE row
        groups 32h) -> one PSUM tile [128, 4, 512], one bank per head
        (concurrent same-bank PE drains are fatal on TRN2).
      - ONE exp ACTIVATE over all 4 heads ([128, 2048] PSUM -> fp16
        SBUF), softmax scale folded into ACT's free affine. No max
        subtraction needed: logits are ~N(0,1).
      - AV: 4 col-packed fp16 matmuls (M=32 at col groups 32h)
        accumulating over key chunks; heads land on disjoint partition
        strips, giving exactly the channel layout proj wants.
      - Softmax denominators: 4 col-packed matmuls against an all-ones
        [128, 32] lhsT -> each head's key-sum replicated over its
        32-partition strip; one reciprocal + one multiply normalizes
        AV with no cross-partition ops.
  * proj in f32r + per-channel bias, DMA out.
Tiles/DMAs are split fine-grained (per head-group / key-chunk) so the
Tile scheduler overlaps DMA, qkv, attention and proj across engines.
"""

import numpy as np

import concourse.bacc as bacc
import concourse.mybir as mybir
import concourse.tile as tile

F32 = mybir.dt.float32
F32R = mybir.dt.float32r
BF16 = mybir.dt.bfloat16
FP16 = mybir.dt.float16

P = 128
C = 256          # channels
N = 2304         # tokens per image
NQ = 1152        # query tokens per core
D = 32           # head dim
KC = N // P      # 18 key chunks
SCALE = D ** -0.5
QT = [(0, 512), (512, 512)]   # full query tiles; 1024:1152 tail is a merged pass
NT3 = 384        # free-dim tile for qkv/proj matmuls (1152 = 3*384)


def emit(tc):
    from contextlib import ExitStack
    ctx = ExitStack()
    nc = tc.nc
    xq_d = nc.dram_tensor("xq", [C, NQ], F32R, kind="ExternalInput").ap()
    xf_d = nc.dram_tensor("xf", [C, N], F32R, kind="ExternalInput").ap()
    wqkvT_d = nc.dram_tensor("wqkvT", [C, 3 * C], F32R, kind="ExternalInput").ap()
    wprojT_d = nc.dram_tensor("wprojT", [C, C], F32R, kind="ExternalInput").ap()
    bprojT_d = nc.dram_tensor("bprojT", [P, 2], F32, kind="ExternalInput").ap()
    y_d = nc.dram_tensor("y", [C, NQ], F32, kind="ExternalOutput").ap()

    singles = ctx.enter_context(tc.tile_pool(name="singles", bufs=1))
    acts = ctx.enter_context(tc.tile_pool(name="acts", bufs=1))
    qkv_ps = ctx.enter_context(tc.tile_pool(name="qkv_ps", bufs=2, space="PSUM"))
    st_ps = ctx.enter_context(tc.tile_pool(name="st_ps", bufs=1, space="PSUM"))
    av_ps = ctx.enter_context(tc.tile_pool(name="av_ps", bufs=1, space="PSUM"))
    sm_ps = ctx.enter_context(tc.tile_pool(name="sm_ps", bufs=1, space="PSUM"))
    pt_pool = ctx.enter_context(tc.tile_pool(name="pt", bufs=3))
    small = ctx.enter_context(tc.tile_pool(name="small", bufs=2))

    # preload the exp table while DMAs/qkv run
    warm = singles.tile([P, 8], F32)
    nc.vector.memset(warm[:], 0.0)
    warm2 = singles.tile([P, 8], F32)
    nc.scalar.activation(warm2[:], warm[:], mybir.ActivationFunctionType.Exp)

    ones_sb = singles.tile([P, D], FP16)
    nc.vector.memset(ones_sb[:], 1.0)
    bias_sb = singles.tile([P, 2], F32)
    nc.sync.dma_start(bias_sb[:], bprojT_d)

    # weights: per-ki-chunk DMAs for early starts
    wq_sb = singles.tile([P, 2, 3 * C], F32R)
    wqkvT_r = wqkvT_d.rearrange("(ki p) o -> p ki o", p=P)
    for ki in range(2):
        nc.sync.dma_start(wq_sb[:, ki, :], wqkvT_r[:, ki, :])
    wp_sb = singles.tile([P, 2, C], F32R)
    nc.sync.dma_start(wp_sb[:], wprojT_d.rearrange("(ki p) o -> p ki o", p=P))

    # x: query half and full image, split by (ki, token range)
    xq_sb = singles.tile([P, 2, NQ], F32R)
    xq_r = xq_d.rearrange("(ki p) n -> p ki n", p=P)
    for ki in range(2):
        for nt in range(NQ // NT3):
            sl = slice(nt * NT3, (nt + 1) * NT3)
            nc.sync.dma_start(xq_sb[:, ki, sl], xq_r[:, ki, sl])
    xf_sb = singles.tile([P, 2, N], F32R)
    xf_r = xf_d.rearrange("(ki p) n -> p ki n", p=P)
    for ki in range(2):
        for nt in range(N // NT3):
            sl = slice(nt * NT3, (nt + 1) * NT3)
            nc.sync.dma_start(xf_sb[:, ki, sl], xf_r[:, ki, sl])

    # per-group activations (separate tiles => fine-grained deps)
    q_g = [acts.tile([P, NQ], F32R, name=f"q{g}") for g in range(2)]
    k_g = [acts.tile([P, N], F32R, name=f"k{g}") for g in range(2)]
    vT_c = [acts.tile([P, C], FP16, name=f"vt{mo}") for mo in range(KC)]
    av_sb = acts.tile([P, 2, NQ], F32R)
    y_sb = acts.tile([P, 2, NQ], F32)

    mm = nc.tensor.matmul

    def qkv_mm(dst_tile, w_col0, rhs_sb, nt):
        sl = slice(nt * NT3, (nt + 1) * NT3)
        ps = qkv_ps.tile([P, NT3], F32, tag="qkv")
        for ki in range(2):
            mm(ps[:], wq_sb[:, ki, w_col0:w_col0 + P], rhs_sb[:, ki, sl],
               start=(ki == 0), stop=(ki == 1))
        nc.vector.tensor_copy(dst_tile[:, sl], ps[:])

    def emit_qkv_group(g):
        # q rows for group g = channels 128g..128g+127; k = 256+128g..
        for nt in range(NQ // NT3):
            qkv_mm(q_g[g], g * P, xq_sb, nt)
        for nt in range(N // NT3):
            qkv_mm(k_g[g], C + g * P, xf_sb, nt)

    def emit_vt(mo):
        ps = qkv_ps.tile([P, NT3], F32, tag="qkv")
        for ki in range(2):
            mm(ps[:, :C], xf_sb[:, ki, mo * P:(mo + 1) * P],
               wq_sb[:, ki, 2 * C:3 * C],
               start=(ki == 0), stop=(ki == 1))
        nc.vector.tensor_copy(vT_c[mo][:], ps[:, :C])

    def emit_attention(g):
        for (q0, qtw) in QT:
            av = av_ps.tile([P, 512], F32)
            sm = sm_ps.tile([P, 512], F32)
            for kc in range(KC):
                st = st_ps.tile([P, 4, 512], F32)
                for h in range(4):
                    mm(st[:, h, :qtw],
                       k_g[g][32 * h:32 * (h + 1), kc * P:(kc + 1) * P],
                       q_g[g][32 * h:32 * (h + 1), q0:q0 + qtw],
                       start=True, stop=True,
                       tile_position=(32 * h, 0))
                pt = pt_pool.tile([P, 4, 512], FP16)
                nc.scalar.activation(pt[:, :, :qtw], st[:, :, :qtw],
                                     mybir.ActivationFunctionType.Exp,
                                     scale=SCALE)
                for h in range(4):
                    mm(av[32 * h:32 * (h + 1), :qtw],
                       vT_c[kc][:, 128 * g + 32 * h:128 * g + 32 * (h + 1)],
                       pt[:, h, :qtw],
                       start=(kc == 0), stop=(kc == KC - 1),
                       tile_position=(0, 32 * h), skip_group_check=True)
                for h in range(4):
                    mm(sm[32 * h:32 * (h + 1), :qtw],
                       ones_sb[:, :],
                       pt[:, h, :qtw],
                       start=(kc == 0), stop=(kc == KC - 1),
                       tile_position=(0, 32 * h), skip_group_check=True)
            rec = small.tile([P, 512], F32, tag="rec")
            nc.vector.reciprocal(rec[:, :qtw], sm[:, :qtw])
            nc.vector.tensor_mul(av_sb[:, g, q0:q0 + qtw], av[:, :qtw],
                                 rec[:, :qtw])

    def emit_tail():
        # queries 1024:1152 for BOTH groups in one pass: head bank h holds
        # g0 at cols 0:128, g1 at cols 128:256. Same-row-group matmuls into
        # one bank serialize on the PE (same cells), so no concurrent
        # same-bank drains.
        q0, qtw = 1024, 128
        av = av_ps.tile([P, 512], F32)
        sm = sm_ps.tile([P, 512], F32)
        for kc in range(KC):
            st = st_ps.tile([P, 4, 512], F32)
            for g in range(2):
                for h in range(4):
                    mm(st[:, h, g * qtw:(g + 1) * qtw],
                       k_g[g][32 * h:32 * (h + 1), kc * P:(kc + 1) * P],
                       q_g[g][32 * h:32 * (h + 1), q0:q0 + qtw],
                       start=(g == 0), stop=(g == 1),
                       tile_position=(32 * h, 0), skip_group_check=True)
            pt = pt_pool.tile([P, 4, 512], FP16)
            nc.scalar.activation(pt[:, :, :2 * qtw], st[:, :, :2 * qtw],
                                 mybir.ActivationFunctionType.Exp,
                                 scale=SCALE)
            for g in range(2):
                for h in range(4):
                    mm(av[32 * h:32 * (h + 1), g * qtw:(g + 1) * qtw],
                       vT_c[kc][:, 128 * g + 32 * h:128 * g + 32 * (h + 1)],
                       pt[:, h, g * qtw:(g + 1) * qtw],
                       start=(kc == 0 and g == 0), stop=(kc == KC - 1 and g == 1),
                       tile_position=(0, 32 * h), skip_group_check=True)
            for g in range(2):
                for h in range(4):
                    mm(sm[32 * h:32 * (h + 1), g * qtw:(g + 1) * qtw],
                       ones_sb[:, :],
                       pt[:, h, g * qtw:(g + 1) * qtw],
                       start=(kc == 0 and g == 0), stop=(kc == KC - 1 and g == 1),
                       tile_position=(0, 32 * h), skip_group_check=True)
        rec = small.tile([P, 512], F32, tag="rec")
        nc.vector.reciprocal(rec[:, :2 * qtw], sm[:, :2 * qtw])
        for g in range(2):
            nc.vector.tensor_mul(av_sb[:, g, q0:q0 + qtw],
                                 av[:, g * qtw:(g + 1) * qtw],
                                 rec[:, g * qtw:(g + 1) * qtw])

    # emission order shapes Tile's priorities: group A's inputs first so
    # the first exp lands as early as possible.
    emit_qkv_group(0)
    for mo in range(KC):
        emit_vt(mo)
    emit_attention(0)
    emit_qkv_group(1)
    emit_attention(1)
    emit_tail()

    # ---- proj ----
    for co in range(2):
        for nt in range(NQ // NT3):
            sl = slice(nt * NT3, (nt + 1) * NT3)
            ps = qkv_ps.tile([P, NT3], F32, tag="qkv")
            for ki in range(2):
                mm(ps[:], wp_sb[:, ki, co * P:(co + 1) * P],
                   av_sb[:, ki, sl],
                   start=(ki == 0), stop=(ki == 1))
            nc.vector.tensor_scalar_add(y_sb[:, co, sl], ps[:],
                                        bias_sb[:, co:co + 1])
    nc.sync.dma_start(y_d.rearrange("(co p) n -> p co n", p=P), y_sb[:])
    ctx.close()


_NC_CACHE = None


def build_nc():
    global _NC_CACHE
    if _NC_CACHE is None:
        nc = bacc.Bacc("TRN2", target_bir_lowering=False, debug=False,
                       num_devices=8)
        with tile.TileContext(nc) as tc:
            emit(tc)
        nc.compile()
        _NC_CACHE = nc
    return _NC_CACHE


def make_in_maps(x, w_qkv, w_proj, b_proj):
    x = np.ascontiguousarray(np.asarray(x, np.float32)).reshape(4, C, N)
    wqkvT = np.ascontiguousarray(np.asarray(w_qkv, np.float32).T)
    wprojT = np.ascontiguousarray(np.asarray(w_proj, np.float32).T)
    bprojT = np.ascontiguousarray(np.asarray(b_proj, np.float32).reshape(2, P).T)
    in_maps = []
    for core in range(8):
        b, t = divmod(core, 2)
        in_maps.append({
            "xq": np.ascontiguousarray(x[b][:, t * NQ:(t + 1) * NQ]),
            "xf": x[b],
            "wqkvT": wqkvT,
            "wprojT": wprojT,
            "bprojT": bprojT,
        })
    return in_maps


def assemble_output(results):
    y = np.empty((4, C, N), np.float32)
    for core in range(8):
        b, t = divmod(core, 2)
        y[b][:, t * NQ:(t + 1) * NQ] = results[core]["y"]
    return y.reshape(4, C, 48, 48)


def kernel(x, w_qkv, w_proj, b_proj):
    from concourse.bass_utils import run_bass_kernel_spmd
    nc = build_nc()
    in_maps = make_in_maps(x, w_qkv, w_proj, b_proj)
    res = run_bass_kernel_spmd(nc, in_maps, core_ids=list(range(8)))
    return assemble_output(res.results)


# revision 17
# speedup vs baseline: 2.5207x; 2.5207x over previous
"""Trainium2 Bass kernel for nn_Attention_40716289966507.

Reference computation (B=4, C=256, H=W=48, heads=8, d=32, N=H*W=2304):
    qkv = w_qkv @ x            # 1x1 conv -> q,k,v each [B, 256, N]
    attn = softmax(q^T k / sqrt(d))   per (batch, head): [N, N]
    out  = v @ attn^T          # [B, 256, N]
    y    = w_proj @ out + b    # [B, 256, N]

Sharding (8 cores): core i handles batch b = i//2 and query-token half
t = i%2 (1152 of the 2304 tokens). Each core needs the full image of its
batch (for K and V) but only its token half for Q; it produces the full
256-channel output for its 1152 tokens, so the host just concatenates —
no cross-core reduction.

Per-core device pipeline (engine-parallel, ACT-bound by design):
  * qkv matmuls in float32r (full-rate fp32-ish); q/k stay f32r for the
    logit matmul's precision, v is materialized TRANSPOSED ([tokens,
    channels], fp16) directly off the tensor engine so the AV matmul
    needs no runtime transposes.
  * Attention per 4-head group, per query tile (512/512/128), per
    128-key chunk:
      - S^T[keys, q] via 4 row-packed f32r matmuls (K=32 at PE row
        groups 32h) -> one PSUM tile [128, 4, 512], one bank per head
        (concurrent same-bank PE drains are fatal on TRN2).
      - ONE exp ACTIVATE over all 4 heads ([128, 2048] PSUM -> fp16
        SBUF), softmax scale folded into ACT's free affine. No max
        subtraction needed: logits are ~N(0,1).
      - AV: 4 col-packed fp16 matmuls (M=32 at col groups 32h)
        accumulating over key chunks; heads land on disjoint partition
        strips, giving exactly the channel layout proj wants.
      - Softmax denominators: 4 col-packed matmuls against an all-ones
        [128, 32] lhsT -> each head's key-sum replicated over its
        32-partition strip; one reciprocal + one multiply normalizes
        AV with no cross-partition ops.
  * proj in f32r + per-channel bias, DMA out.
Tiles/DMAs are split fine-grained (per head-group / key-chunk) so the
Tile scheduler overlaps DMA, qkv, attention and proj across engines.
"""

import numpy as np

import concourse.bacc as bacc
import concourse.mybir as mybir
import concourse.tile as tile

F32 = mybir.dt.float32
F32R = mybir.dt.float32r
BF16 = mybir.dt.bfloat16
FP16 = mybir.dt.float16

P = 128
C = 256          # channels
N = 2304         # tokens per image
NQ = 1152        # query tokens per core
D = 32           # head dim
KC = N // P      # 18 key chunks
SCALE = D ** -0.5
QT = [(0, 512), (512, 512)]   # full query tiles; 1024:1152 tail is a merged pass
NT3 = 384        # free-dim tile for qkv/proj matmuls (1152 = 3*384)


def emit(tc):
    from contextlib import ExitStack
    ctx = ExitStack()
    nc = tc.nc
    xq_d = nc.dram_tensor("xq", [C, NQ], F32R, kind="ExternalInput").ap()
    xf_d = nc.dram_tensor("xf", [C, N], F32R, kind="ExternalInput").ap()
    wqkvT_d = nc.dram_tensor("wqkvT", [C, 3 * C], F32R, kind="ExternalInput").ap()
    wprojT_d = nc.dram_tensor("wprojT", [C, C], F32R, kind="ExternalInput").ap()
    bprojT_d = nc.dram_tensor("bprojT", [P, 2], F32, kind="ExternalInput").ap()
    y_d = nc.dram_tensor("y", [C, NQ], F32, kind="ExternalOutput").ap()

    singles = ctx.enter_context(tc.tile_pool(name="singles", bufs=1))
    acts = ctx.enter_context(tc.tile_pool(name="acts", bufs=1))
    qkv_ps = ctx.enter_context(tc.tile_pool(name="qkv_ps", bufs=2, space="PSUM"))
    st_ps = ctx.enter_context(tc.tile_pool(name="st_ps", bufs=1, space="PSUM"))
    av_ps = ctx.enter_context(tc.tile_pool(name="av_ps", bufs=1, space="PSUM"))
    sm_ps = ctx.enter_context(tc.tile_pool(name="sm_ps", bufs=1, space="PSUM"))
    pt_pool = ctx.enter_context(tc.tile_pool(name="pt", bufs=3))
    small = ctx.enter_context(tc.tile_pool(name="small", bufs=2))

    # preload the exp table while DMAs/qkv run
    warm = singles.tile([P, 8], F32)
    nc.vector.memset(warm[:], 0.0)
    warm2 = singles.tile([P, 8], F32)
    nc.scalar.activation(warm2[:], warm[:], mybir.ActivationFunctionType.Exp)

    ones_sb = singles.tile([P, D], FP16)
    nc.vector.memset(ones_sb[:], 1.0)
    bias_sb = singles.tile([P, 2], F32)
    nc.sync.dma_start(bias_sb[:], bprojT_d)

    # weights: per-ki-chunk DMAs for early starts
    wq_sb = singles.tile([P, 2, 3 * C], F32R)
    wqkvT_r = wqkvT_d.rearrange("(ki p) o -> p ki o", p=P)
    for ki in range(2):
        nc.sync.dma_start(wq_sb[:, ki, :], wqkvT_r[:, ki, :])
    wp_sb = singles.tile([P, 2, C], F32R)
    nc.sync.dma_start(wp_sb[:], wprojT_d.rearrange("(ki p) o -> p ki o", p=P))

    # x: query half and full image, split by (ki, token range)
    xq_sb = singles.tile([P, 2, NQ], F32R)
    xq_r = xq_d.rearrange("(ki p) n -> p ki n", p=P)
    for ki in range(2):
        for nt in range(NQ // NT3):
            sl = slice(nt * NT3, (nt + 1) * NT3)
            nc.sync.dma_start(xq_sb[:, ki, sl], xq_r[:, ki, sl])
    xf_sb = singles.tile([P, 2, N], F32R)
    xf_r = xf_d.rearrange("(ki p) n -> p ki n", p=P)
    for ki in range(2):
        for nt in range(N // NT3):
            sl = slice(nt * NT3, (nt + 1) * NT3)
            nc.sync.dma_start(xf_sb[:, ki, sl], xf_r[:, ki, sl])

    # per-group activations (separate tiles => fine-grained deps)
    q_g = [acts.tile([P, NQ], F32R, name=f"q{g}") for g in range(2)]
    k_g = [acts.tile([P, N], F32R, name=f"k{g}") for g in range(2)]
    vT_c = [acts.tile([P, C], FP16, name=f"vt{mo}") for mo in range(KC)]
    av_sb = acts.tile([P, 2, NQ], F32R)
    y_sb = acts.tile([P, 2, NQ], F32)

    mm = nc.tensor.matmul

    def qkv_mm(dst_tile, w_col0, rhs_sb, nt):
        sl = slice(nt * NT3, (nt + 1) * NT3)
        ps = qkv_ps.tile([P, NT3], F32, tag="qkv")
        for ki in range(2):
            mm(ps[:], wq_sb[:, ki, w_col0:w_col0 + P], rhs_sb[:, ki, sl],
               start=(ki == 0), stop=(ki == 1))
        nc.vector.tensor_copy(dst_tile[:, sl], ps[:])

    def emit_qkv_group(g):
        # q rows for group g = channels 128g..128g+127; k = 256+128g..
        for nt in range(NQ // NT3):
            qkv_mm(q_g[g], g * P, xq_sb, nt)
        for nt in range(N // NT3):
            qkv_mm(k_g[g], C + g * P, xf_sb, nt)

    def emit_vt(mo):
        ps = qkv_ps.tile([P, NT3], F32, tag="qkv")
        for ki in range(2):
            mm(ps[:, :C], xf_sb[:, ki, mo * P:(mo + 1) * P],
               wq_sb[:, ki, 2 * C:3 * C],
               start=(ki == 0), stop=(ki == 1))
        nc.vector.tensor_copy(vT_c[mo][:], ps[:, :C])

    def emit_attention(g):
        for (q0, qtw) in QT:
            av = av_ps.tile([P, 512], F32)
            sm = sm_ps.tile([P, 512], F32)
            for kc in range(KC):
                st = st_ps.tile([P, 4, 512], F32)
                for h in range(4):
                    mm(st[:, h, :qtw],
                       k_g[g][32 * h:32 * (h + 1), kc * P:(kc + 1) * P],
                       q_g[g][32 * h:32 * (h + 1), q0:q0 + qtw],
                       start=True, stop=True,
                       tile_position=(32 * h, 0))
                pt = pt_pool.tile([P, 4, 512], FP16)
                nc.scalar.activation(pt[:, :, :qtw], st[:, :, :qtw],
                                     mybir.ActivationFunctionType.Exp,
                                     scale=SCALE)
                for h in range(4):
                    mm(av[32 * h:32 * (h + 1), :qtw],
                       vT_c[kc][:, 128 * g + 32 * h:128 * g + 32 * (h + 1)],
                       pt[:, h, :qtw],
                       start=(kc == 0), stop=(kc == KC - 1),
                       tile_position=(0, 32 * h), skip_group_check=True)
                for h in range(4):
                    mm(sm[32 * h:32 * (h + 1), :qtw],
                       ones_sb[:, :],
                       pt[:, h, :qtw],
                       start=(kc == 0), stop=(kc == KC - 1),
                       tile_position=(0, 32 * h), skip_group_check=True)
            rec = small.tile([P, 512], F32, tag="rec")
            nc.vector.reciprocal(rec[:, :qtw], sm[:, :qtw])
            nc.vector.tensor_mul(av_sb[:, g, q0:q0 + qtw], av[:, :qtw],
                                 rec[:, :qtw])

    def emit_tail():
        # queries 1024:1152 for BOTH groups in one pass: head bank h holds
        # g0 at cols 0:128, g1 at cols 128:256. Same-row-group matmuls into
        # one bank serialize on the PE (same cells), so no concurrent
        # same-bank drains.
        q0, qtw = 1024, 128
        av = av_ps.tile([P, 512], F32)
        sm = sm_ps.tile([P, 512], F32)
        for kc in range(KC):
            st = st_ps.tile([P, 4, 512], F32)
            for g in range(2):
                for h in range(4):
                    mm(st[:, h, g * qtw:(g + 1) * qtw],
                       k_g[g][32 * h:32 * (h + 1), kc * P:(kc + 1) * P],
                       q_g[g][32 * h:32 * (h + 1), q0:q0 + qtw],
                       start=(g == 0), stop=(g == 1),
                       tile_position=(32 * h, 0), skip_group_check=True)
            pt = pt_pool.tile([P, 4, 512], FP16)
            nc.scalar.activation(pt[:, :, :2 * qtw], st[:, :, :2 * qtw],
                                 mybir.ActivationFunctionType.Exp,
                                 scale=SCALE)
            for g in range(2):
                for h in range(4):
                    mm(av[32 * h:32 * (h + 1), g * qtw:(g + 1) * qtw],
                       vT_c[kc][:, 128 * g + 32 * h:128 * g + 32 * (h + 1)],
                       pt[:, h, g * qtw:(g + 1) * qtw],
                       start=(kc == 0 and g == 0), stop=(kc == KC - 1 and g == 1),
                       tile_position=(0, 32 * h), skip_group_check=True)
            for g in range(2):
                for h in range(4):
                    mm(sm[32 * h:32 * (h + 1), g * qtw:(g + 1) * qtw],
                       ones_sb[:, :],
                       pt[:, h, g * qtw:(g + 1) * qtw],
                       start=(kc == 0 and g == 0), stop=(kc == KC - 1 and g == 1),
                       tile_position=(0, 32 * h), skip_group_check=True)
        rec = small.tile([P, 512], F32, tag="rec")
        nc.vector.reciprocal(rec[:, :2 * qtw], sm[:, :2 * qtw])
        for g in range(2):
            nc.vector.tensor_mul(av_sb[:, g, q0:q0 + qtw],
                                 av[:, g * qtw:(g + 1) * qtw],
                                 rec[:, g * qtw:(g + 1) * qtw])

    y_r = y_d.rearrange("(co p) n -> p co n", p=P)

    def emit_proj(co, nt):
        sl = slice(nt * NT3, (nt + 1) * NT3)
        ps = qkv_ps.tile([P, NT3], F32, tag="qkv")
        for ki in range(2):
            mm(ps[:], wp_sb[:, ki, co * P:(co + 1) * P],
               av_sb[:, ki, sl],
               start=(ki == 0), stop=(ki == 1))
        nc.vector.tensor_scalar_add(y_sb[:, co, sl], ps[:],
                                    bias_sb[:, co:co + 1])
        nc.sync.dma_start(y_r[:, co, sl], y_sb[:, co, sl])

    # emission order shapes Tile's priorities: group A's inputs first so
    # the first exp lands as early as possible. proj chunks for queries
    # 0:768 only need av_sb written by the full query tiles, so they are
    # emitted before the merged tail pass and fill its engine gaps; the
    # last proj chunk (queries 768:1152) follows the tail. Output DMA is
    # per-chunk so results ship while later chunks still compute.
    emit_qkv_group(0)
    for mo in range(KC):
        emit_vt(mo)
    emit_attention(0)
    emit_qkv_group(1)
    emit_attention(1)
    for co in range(2):
        for nt in range(2):
            emit_proj(co, nt)
    emit_tail()
    for co in range(2):
        emit_proj(co, 2)
    ctx.close()


_NC_CACHE = None


def build_nc():
    global _NC_CACHE
    if _NC_CACHE is None:
        nc = bacc.Bacc("TRN2", target_bir_lowering=False, debug=False,
                       num_devices=8)
        with tile.TileContext(nc) as tc:
            emit(tc)
        nc.compile()
        _NC_CACHE = nc
    return _NC_CACHE


def make_in_maps(x, w_qkv, w_proj, b_proj):
    x = np.ascontiguousarray(np.asarray(x, np.float32)).reshape(4, C, N)
    wqkvT = np.ascontiguousarray(np.asarray(w_qkv, np.float32).T)
    wprojT = np.ascontiguousarray(np.asarray(w_proj, np.float32).T)
    bprojT = np.ascontiguousarray(np.asarray(b_proj, np.float32).reshape(2, P).T)
    in_maps = []
    for core in range(8):
        b, t = divmod(core, 2)
        in_maps.append({
            "xq": np.ascontiguousarray(x[b][:, t * NQ:(t + 1) * NQ]),
            "xf": x[b],
            "wqkvT": wqkvT,
            "wprojT": wprojT,
            "bprojT": bprojT,
        })
    return in_maps


def assemble_output(results):
    y = np.empty((4, C, N), np.float32)
    for core in range(8):
        b, t = divmod(core, 2)
        y[b][:, t * NQ:(t + 1) * NQ] = results[core]["y"]
    return y.reshape(4, C, 48, 48)


def kernel(x, w_qkv, w_proj, b_proj):
    from concourse.bass_utils import run_bass_kernel_spmd
    nc = build_nc()
    in_maps = make_in_maps(x, w_qkv, w_proj, b_proj)
    res = run_bass_kernel_spmd(nc, in_maps, core_ids=list(range(8)))
    return assemble_output(res.results)
